# revision 11
# baseline (speedup 1.0000x reference)
"""Contrastive-learning loss kernel for Trainium2 (8 NeuronCores, Bass/Tile).

Problem (hardcoded shapes): B=16, L=512, DIN1=256, DIN2=192, DH=256, DF=128.
  emb1 = MLP_a(feature1); emb2 = MLP_b(feature2)          # (B, L, DF)
  positive = rowdot(f1, f2) + band-mean terms              # (N,)  N = B*L = 8192
  negative = logsumexp(f1 @ f2.T, axis=-1) - log N         # (N,)
  loss = mean(-positive + negative)

Sharding: data-parallel over B for embeddings/positives (2 batches per core);
the N x N negatives matrix is sharded row-wise. Each core computes the full
emb2 from a column-ROTATED copy of feature2 (its own batches first), so the
device program is identical across cores (pure SPMD, no partition-id): the
local rows are always columns [0, 1024) and logsumexp is invariant to column
order.

Schedule notes:
  - ScalarE exp throughput (~64 x 1.25us) is the critical path; everything
    else hides under it.
  - dma_start issue on the Sync queue costs ~1.4us each, so all weights,
    biases and 1/count tables ship as ONE packed bf16 tensor (f32 fields are
    bitcast views); x1t is one DMA; x2 is four. Band masks load after x2.
  - The chain to the first exp is minimal: MLP1 chunk 0 -> MLP2 chunks 0/1
    -> negative matmuls tile 0. Startup relu/bias run on the (otherwise
    idle) ScalarE in parallel with VectorE. MLP1 chunk 1 hides between the
    first negative tiles; transposes/bands/positives spread across the
    8-group loop where PE/DVE have slack.

Outputs per core: pos_out (128, 8), se_out (128, 8) where column t holds
local rows [t*128, (t+1)*128). Host: loss = mean(-pos + log(se) - log N).
"""

import numpy as np

import concourse.bacc as bacc
import concourse.tile as tile
from concourse import mybir
from concourse.bass_utils import run_bass_kernel_spmd
from concourse.masks import make_identity

F32 = mybir.dt.float32
F32R = mybir.dt.float32r
BF16 = mybir.dt.bfloat16

B, L, DIN1, DIN2, DH, DF = 16, 512, 256, 192, 256, 128
N = B * L            # 8192 total rows
NCORES = 8
NB = B // NCORES     # 2 local batches per core
NLOC = NB * L        # 1024 local rows per core
NT = NLOC // 128     # 8 local row tiles
NEG_FD = 1024        # columns exp'd per activation instruction
NGRP = N // NEG_FD   # 8 column groups

# packed-constants column offsets (bf16 columns)
OW1A, OW2A, OW2B, OW1BA, OW1BB = 0, 512, 768, 1024, 1280
OB1A, OB2A, OB1B, OB2B, OCIS, OCIT = 1536, 1540, 1542, 1546, 1548, 1564
CW = 1580


def _build(share_tgt: bool):
    nc = bacc.Bacc("TRN2", target_bir_lowering=False, debug=False)

    x1t_d = nc.dram_tensor("x1t", [DIN1, NLOC], BF16, kind="ExternalInput")
    x2t_d = nc.dram_tensor("x2t", [DIN2, N], BF16, kind="ExternalInput")
    wpk_d = nc.dram_tensor("wpk", [128, CW], BF16, kind="ExternalInput")
    bms_d = nc.dram_tensor("bms", [L, L], BF16, kind="ExternalInput")
    if not share_tgt:
        bmt_d = nc.dram_tensor("bmt", [L, L], BF16, kind="ExternalInput")
    pos_d = nc.dram_tensor("pos_out", [128, NT], F32, kind="ExternalOutput")
    se_d = nc.dram_tensor("se_out", [128, NT], F32, kind="ExternalOutput")

    with tile.TileContext(nc) as tc:
        import contextlib

        with contextlib.ExitStack() as stack:
            const = stack.enter_context(tc.tile_pool(name="const", bufs=1))
            big = stack.enter_context(tc.tile_pool(name="big", bufs=1))
            h2pool = stack.enter_context(tc.tile_pool(name="h2pool", bufs=3))
            posp = stack.enter_context(tc.tile_pool(name="posp", bufs=2))

            # ---- inputs: few, large DMAs, critical-path first ----
            x1t = big.tile([128, 2, NLOC], BF16)
            nc.sync.dma_start(
                out=x1t[:], in_=x1t_d.ap().rearrange("(t p) c -> p t c", p=128)
            )
            wpk = const.tile([128, CW], BF16)
            nc.sync.dma_start(out=wpk[:], in_=wpk_d.ap())
            x2a = big.tile([128, N], BF16)
            x2b = big.tile([64, N], BF16)
            nc.sync.dma_start(out=x2a[:, 0:1024], in_=x2t_d.ap()[0:128, 0:1024])
            nc.sync.dma_start(out=x2b[:, 0:1024], in_=x2t_d.ap()[128:DIN2, 0:1024])
            nc.sync.dma_start(out=x2a[:, 1024:4096], in_=x2t_d.ap()[0:128, 1024:4096])
            nc.sync.dma_start(out=x2b[:, 1024:4096], in_=x2t_d.ap()[128:DIN2, 1024:4096])
            nc.sync.dma_start(out=x2a[:, 4096:N], in_=x2t_d.ap()[0:128, 4096:N])
            nc.sync.dma_start(out=x2b[:, 4096:N], in_=x2t_d.ap()[128:DIN2, 4096:N])
            bms = const.tile([128, 4, L], BF16)
            nc.sync.dma_start(
                out=bms[:], in_=bms_d.ap().rearrange("(t p) j -> p t j", p=128)
            )
            if share_tgt:
                bmt = bms
            else:
                bmt = const.tile([128, 4, L], BF16)
                nc.sync.dma_start(
                    out=bmt[:], in_=bmt_d.ap().rearrange("(t p) j -> p t j", p=128)
                )

            def b1a(mt):
                return wpk[:, OB1A + 2 * mt : OB1A + 2 * mt + 2].bitcast(F32)

            def b1b(mt):
                return wpk[:, OB1B + 2 * mt : OB1B + 2 * mt + 2].bitcast(F32)

            b2a = wpk[:, OB2A : OB2A + 2].bitcast(F32)
            b2b = wpk[:, OB2B : OB2B + 2].bitcast(F32)
            cis = wpk[:, OCIS : OCIS + 16].bitcast(F32)
            cit = cis if share_tgt else wpk[:, OCIT : OCIT + 16].bitcast(F32)

            ident = const.tile([128, 128], F32)
            make_identity(nc, ident[:])
            ident_bf = const.tile([128, 128], BF16)
            nc.vector.tensor_copy(ident_bf[:], ident[:])

            e1t = big.tile([128, NLOC], BF16)
            e2t = big.tile([128, N], BF16)
            h1t = big.tile([128, 2, NLOC], BF16)
            e1nat = big.tile([128, NT, DF], BF16)
            e2nat = big.tile([128, NT, DF], BF16)
            w1nat = big.tile([128, NT, DF], F32)
            w2snat = big.tile([128, NT, DF], F32)
            w2tnat = w2snat if share_tgt else big.tile([128, NT, DF], F32)
            pos_all = big.tile([128, NT], F32)
            acc_all = big.tile([128, NT * NGRP + 1], F32)
            se_all = big.tile([128, NT], F32)

            psA = stack.enter_context(tc.tile_pool(name="psumA", bufs=1, space="PSUM"))

            # ---- dummy exp to prefetch the activation table during DMA
            scr = const.tile([128, 8], F32)
            nc.scalar.activation(
                out=scr[:],
                in_=wpk[:, 0:8],
                func=mybir.ActivationFunctionType.Exp,
            )

            def relu_bias(dst, src, bias, use_scalar):
                if use_scalar:
                    nc.scalar.activation(
                        out=dst,
                        in_=src,
                        func=mybir.ActivationFunctionType.Relu,
                        bias=bias,
                    )
                else:
                    nc.vector.tensor_scalar(
                        out=dst,
                        in0=src,
                        scalar1=bias,
                        scalar2=0.0,
                        op0=mybir.AluOpType.add,
                        op1=mybir.AluOpType.max,
                    )

            def add_bias(dst, src, bias, use_scalar):
                if use_scalar:
                    nc.scalar.activation(
                        out=dst,
                        in_=src,
                        func=mybir.ActivationFunctionType.Identity,
                        bias=bias,
                    )
                else:
                    nc.vector.tensor_scalar_add(out=dst, in0=src, scalar1=bias)

            # ---- MLP1: h1T = relu(W1a^T @ x1T + b1a); e1T = W2a^T @ h1T + b2a
            def mlp1_h(cols, sa=False):
                w = cols.stop - cols.start
                h1ps_full = psA.tile([128, 2, 512], F32, tag="hps", bufs=1,
                                     name=f"h1ps{cols.start}")
                h1ps = h1ps_full[:, :, 0:w]
                for mt in range(2):
                    for kt in range(2):
                        nc.tensor.matmul(
                            h1ps[:, mt, :],
                            wpk[:, OW1A + kt * 256 + mt * 128 : OW1A + kt * 256 + (mt + 1) * 128],
                            x1t[:, kt, cols],
                            start=(kt == 0),
                            stop=(kt == 1),
                        )
                for mt in range(2):
                    relu_bias(h1t[:, mt, cols], h1ps[:, mt, :], b1a(mt), sa and mt == 0)

            def mlp1_e(cols, sa=False):
                w = cols.stop - cols.start
                e1ps_full = psA.tile([128, 512], F32, tag="sps", bufs=2,
                                     name=f"e1ps{cols.start}")
                e1ps = e1ps_full[:, 0:w]
                for kt in range(2):
                    nc.tensor.matmul(
                        e1ps[:],
                        wpk[:, OW2A + kt * 128 : OW2A + (kt + 1) * 128],
                        h1t[:, kt, cols],
                        start=(kt == 0),
                        stop=(kt == 1),
                    )
                add_bias(e1t[:, cols], e1ps[:], b2a, sa)

            def transpose_to(dst, srcT, t):
                tp = psA.tile([128, 128], BF16, tag="sps", bufs=2, name=f"tp{t}")
                nc.tensor.transpose(
                    tp[:], srcT[:, t * 128 : (t + 1) * 128], ident_bf[:]
                )
                nc.vector.tensor_copy(dst[:, t, :], tp[:])

            # banded sums: W_sum[j,:] = sum_{|m-j|<=r} e[m,:]  (bf16 matmuls)
            def band_half(dst, bm, src, half):
                for b in range(NB):
                    for jt in (2 * half, 2 * half + 1):
                        wps = psA.tile([128, 128], F32, tag="sps", bufs=2)
                        for mt in range(4):
                            nc.tensor.matmul(
                                wps[:],
                                bm[:, mt, jt * 128 : (jt + 1) * 128],
                                src[:, 4 * b + mt, :],
                                start=(mt == 0),
                                stop=(mt == 3),
                            )
                        nc.vector.tensor_copy(dst[:, 4 * b + jt, :], wps[:])

            # ---- MLP2 over all N tokens ----
            def mlp2_chunk(ct, sa=False):
                cols = slice(ct * 512, (ct + 1) * 512)
                h2ps = psA.tile([128, 2, 512], F32, tag="hps", bufs=1, name=f"h2ps{ct}")
                for mt in range(2):
                    nc.tensor.matmul(
                        h2ps[:, mt, :],
                        wpk[:, OW1BA + mt * 128 : OW1BA + (mt + 1) * 128],
                        x2a[:, cols],
                        start=True,
                        stop=False,
                    )
                    nc.tensor.matmul(
                        h2ps[:, mt, :],
                        wpk[0:64, OW1BB + mt * 128 : OW1BB + (mt + 1) * 128],
                        x2b[:, cols],
                        start=False,
                        stop=True,
                    )
                h2t = h2pool.tile([128, 2, 512], BF16, tag="h2t", name=f"h2t{ct}")
                for mt in range(2):
                    relu_bias(h2t[:, mt, :], h2ps[:, mt, :], b1b(mt), sa and mt == 0)
                e2ps = psA.tile([128, 512], F32, tag="sps", bufs=2, name=f"e2ps{ct}")
                for kt in range(2):
                    nc.tensor.matmul(
                        e2ps[:],
                        wpk[:, OW2B + kt * 128 : OW2B + (kt + 1) * 128],
                        h2t[:, kt, :],
                        start=(kt == 0),
                        stop=(kt == 1),
                    )
                add_bias(e2t[:, cols], e2ps[:], b2b, sa)

            def neg_half(g, t, h, acc_col):
                lhs = e1t[:, t * 128 : (t + 1) * 128]
                np_full = psA.tile([128, NEG_FD], F32, tag="neg", bufs=2, name="nph")
                np_ps = np_full[:, 0:512]
                c0 = g * NEG_FD + h * 512
                nc.tensor.matmul(
                    np_ps[:], lhs, e2t[:, c0 : c0 + 512], start=True, stop=True
                )
                nc.scalar.activation(
                    out=np_ps[:],
                    in_=np_ps[:],
                    func=mybir.ActivationFunctionType.Exp,
                    accum_out=acc_all[:, acc_col : acc_col + 1],
                )

            def neg_tile(g, t):
                lhs = e1t[:, t * 128 : (t + 1) * 128]
                np_ps = psA.tile([128, NEG_FD], F32, tag="neg", bufs=2)
                for i in range(NEG_FD // 512):
                    c0 = g * NEG_FD + i * 512
                    nc.tensor.matmul(
                        np_ps[:, i * 512 : (i + 1) * 512],
                        lhs,
                        e2t[:, c0 : c0 + 512],
                        start=True,
                        stop=True,
                    )
                idx = t * NGRP + g
                nc.scalar.activation(
                    out=np_ps[:],
                    in_=np_ps[:],
                    func=mybir.ActivationFunctionType.Exp,
                    accum_out=acc_all[:, idx : idx + 1],
                )

            # positives for one local batch b (VectorE only)
            def positives(b):
                bsl = slice(4 * b, 4 * b + 4)
                ga = posp.tile([128, 4, DF], F32, tag="posg")
                r1 = posp.tile([128, 4], F32, tag="post")
                r2 = posp.tile([128, 4], F32, tag="post")
                if share_tgt:
                    nc.vector.tensor_add(ga[:], w1nat[:, bsl, :], w2snat[:, bsl, :])
                    nc.vector.tensor_mul(ga[:], ga[:], e1nat[:, bsl, :])
                else:
                    nc.vector.tensor_mul(ga[:], w1nat[:, bsl, :], e1nat[:, bsl, :])
                nc.vector.tensor_reduce(
                    out=r1[:], in_=ga[:], axis=mybir.AxisListType.X, op=mybir.AluOpType.add
                )
                gb = posp.tile([128, 4, DF], F32, tag="posg")
                nc.vector.tensor_mul(gb[:], w2snat[:, bsl, :], e2nat[:, bsl, :])
                nc.vector.tensor_reduce(
                    out=r2[:], in_=gb[:], axis=mybir.AxisListType.X, op=mybir.AluOpType.add
                )
                nc.vector.tensor_add(r1[:], r1[:], r2[:])
                nc.vector.tensor_mul(r1[:], r1[:], cis[:, bsl])
                if not share_tgt:
                    gc = posp.tile([128, 4, DF], F32, tag="posg")
                    nc.vector.tensor_mul(gc[:], w2tnat[:, bsl, :], e1nat[:, bsl, :])
                    rt = posp.tile([128, 4], F32, tag="post")
                    nc.vector.tensor_reduce(
                        out=rt[:], in_=gc[:], axis=mybir.AxisListType.X,
                        op=mybir.AluOpType.add,
                    )
                    nc.vector.tensor_mul(rt[:], rt[:], cit[:, bsl])
                    nc.vector.tensor_add(r1[:], r1[:], rt[:])
                gd = posp.tile([128, 4, DF], BF16, tag="posgb")
                nc.vector.tensor_mul(gd[:], e1nat[:, bsl, :], e2nat[:, bsl, :])
                r3 = posp.tile([128, 4], F32, tag="post")
                nc.vector.tensor_reduce(
                    out=r3[:], in_=gd[:], axis=mybir.AxisListType.X, op=mybir.AluOpType.add
                )
                nc.vector.tensor_add(pos_all[:, bsl], r1[:], r3[:])

            # per-group deferred work under the exp-bound steady state
            if share_tgt:
                extras = {
                    0: [lambda: [transpose_to(e1nat, e1t, t) for t in range(NT)]],
                    1: [lambda: [transpose_to(e2nat, e2t, t) for t in range(NT)]],
                    2: [lambda: band_half(w1nat, bms, e1nat, 0)],
                    3: [lambda: band_half(w1nat, bms, e1nat, 1)],
                    4: [lambda: band_half(w2snat, bms, e2nat, 0)],
                    5: [lambda: band_half(w2snat, bms, e2nat, 1)],
                    6: [lambda: positives(0)],
                    7: [lambda: positives(1)],
                }
            else:
                extras = {
                    0: [lambda: [transpose_to(e1nat, e1t, t) for t in range(NT)]],
                    1: [lambda: [transpose_to(e2nat, e2t, t) for t in range(NT)]],
                    2: [lambda: band_half(w1nat, bms, e1nat, 0),
                        lambda: band_half(w1nat, bms, e1nat, 1)],
                    3: [lambda: band_half(w2snat, bms, e2nat, 0)],
                    4: [lambda: band_half(w2snat, bms, e2nat, 1)],
                    5: [lambda: band_half(w2tnat, bmt, e2nat, 0)],
                    6: [lambda: band_half(w2tnat, bmt, e2nat, 1)],
                    7: [lambda: positives(0), lambda: positives(1)],
                }

            # ---- minimal-latency chain to the first exp: micro-MLP1 for the
            # first 128 e1 columns, MLP2 chunk 0, then an F=512 exp on row
            # tile 0 (its two halves use accum columns 64 and 0). The rest
            # of MLP1 and the deferred work hide under the exp stream.
            mlp1_h(slice(0, 128), sa=True)
            mlp1_e(slice(0, 128), sa=True)
            mlp2_chunk(0, sa=True)
            neg_half(0, 0, 0, NT * NGRP)
            mlp2_chunk(1, sa=True)
            neg_half(0, 0, 1, 0)
            mlp1_h(slice(128, 512))
            mlp1_e(slice(128, 512))
            neg_tile(0, 1)
            neg_tile(0, 2)
            mlp1_h(slice(512, 1024))
            neg_tile(0, 3)
            mlp1_e(slice(512, 1024))
            neg_tile(0, 4)
            neg_tile(0, 5)
            mlp2_chunk(2)
            neg_tile(0, 6)
            neg_tile(0, 7)
            mlp2_chunk(3)
            for fn in extras[0]:
                fn()
            for g in range(1, NGRP):
                for t in range(4):
                    neg_tile(g, t)
                if g < NGRP - 1:
                    mlp2_chunk(2 * g + 2)
                neg_tile(g, 4)
                neg_tile(g, 5)
                if g < NGRP - 1:
                    mlp2_chunk(2 * g + 3)
                for fn in extras[g]:
                    fn()
                neg_tile(g, 6)
                neg_tile(g, 7)

            nc.sync.dma_start(out=pos_d.ap(), in_=pos_all[:])
            for t in range(NT):
                nc.vector.tensor_reduce(
                    out=se_all[:, t : t + 1],
                    in_=acc_all[:, t * NGRP : (t + 1) * NGRP],
                    axis=mybir.AxisListType.X,
                    op=mybir.AluOpType.add,
                )
            nc.vector.tensor_add(
                se_all[:, 0:1], se_all[:, 0:1], acc_all[:, NT * NGRP : NT * NGRP + 1]
            )
            nc.sync.dma_start(out=se_d.ap(), in_=se_all[:])

    nc.compile()
    return nc


_BUILD_CACHE: dict = {}


def _get_nc(share_tgt: bool):
    if share_tgt not in _BUILD_CACHE:
        _BUILD_CACHE[share_tgt] = _build(share_tgt)
    return _BUILD_CACHE[share_tgt]


def _band_mask(r: int) -> np.ndarray:
    """mask[m, j] = 1 if |m-j| <= r (and inside [0,L)) else 0."""
    bm = np.zeros((L, L), dtype=np.float32)
    if r > 0:
        j = np.arange(L)
        lo = np.maximum(j - r, 0)
        hi = np.minimum(j + r + 1, L)
        m = np.arange(L)[:, None]
        bm = ((m >= lo[None, :]) & (m < hi[None, :])).astype(np.float32)
    return bm


def _cnt_inv(r: int) -> np.ndarray:
    """(128, NT) tile of 1/count(j) per local row (j = row mod L)."""
    j = np.arange(L)
    if r > 0:
        cnt = (np.minimum(j + r + 1, L) - np.maximum(j - r, 0)).astype(np.float64)
    else:
        cnt = np.ones(L)
    cinv = (1.0 / cnt).astype(np.float32)
    rows = (np.arange(NLOC) % L)
    return np.ascontiguousarray(cinv[rows].reshape(NT, 128).T)


def _pack_consts(W1a, W2a, W1b, W2b, b1a, b2a, b1b, b2b, cis, cit):
    """One (128, CW) bf16 tensor holding every weight/bias/count table.
    f32 fields are stored as raw byte pairs (device reads them via bitcast)."""
    import ml_dtypes

    bf16 = ml_dtypes.bfloat16
    pk = np.zeros((128, CW), dtype=np.uint16)

    def put_bf(col, arr):
        a = np.ascontiguousarray(arr.astype(bf16)).view(np.uint16)
        pk[: a.shape[0], col : col + a.shape[1]] = a

    def put_f32(col, arr):
        a = np.ascontiguousarray(arr.astype(np.float32)).view(np.uint16)
        pk[: a.shape[0], col : col + a.shape[1]] = a

    # w1a[p, kt*256 + m] = W1a[kt*128 + p, m]
    put_bf(OW1A, np.asarray(W1a, np.float32).reshape(2, 128, DH).transpose(1, 0, 2).reshape(128, 512))
    put_bf(OW2A, np.asarray(W2a, np.float32).reshape(2, 128, DF).transpose(1, 0, 2).reshape(128, 256))
    put_bf(OW2B, np.asarray(W2b, np.float32).reshape(2, 128, DF).transpose(1, 0, 2).reshape(128, 256))
    w1b = np.asarray(W1b, np.float32)
    put_bf(OW1BA, w1b[0:128, :])
    # w1b_b for mt: partitions 0:64 of cols OW1BB+128*mt = W1b[128+r, 128*mt+c]
    put_bf(OW1BB, w1b[128:192, :].reshape(64, 2, 128).transpose(0, 1, 2).reshape(64, 256))
    put_f32(OB1A, np.asarray(b1a, np.float32).reshape(2, 128).T)
    put_f32(OB2A, np.asarray(b2a, np.float32).reshape(128, 1))
    put_f32(OB1B, np.asarray(b1b, np.float32).reshape(2, 128).T)
    put_f32(OB2B, np.asarray(b2b, np.float32).reshape(128, 1))
    put_f32(OCIS, cis)
    if cit is not None:
        put_f32(OCIT, cit)
    return pk.view(bf16)


def kernel(**inputs):
    loss, _ = _run(inputs, trace=False)
    return loss


def _run(inputs, trace=False, trace_kwargs=None):
    import ml_dtypes

    bf16 = ml_dtypes.bfloat16
    feature1 = inputs["feature1"]
    feature2 = inputs["feature2"]
    W1a, b1a, W2a, b2a = inputs["W1a"], inputs["b1a"], inputs["W2a"], inputs["b2a"]
    W1b, b1b, W2b, b2b = inputs["W1b"], inputs["b1b"], inputs["W2b"], inputs["b2b"]
    f1 = np.ascontiguousarray(np.asarray(feature1, dtype=np.float32))
    f2 = np.ascontiguousarray(np.asarray(feature2, dtype=np.float32))
    r_self = int(np.asarray(inputs["positive_range_self"]))
    r_tgt = int(np.asarray(inputs["positive_range_tgt"]))
    share_tgt = r_tgt == r_self

    nc = _get_nc(share_tgt)

    x2t_full = np.ascontiguousarray(f2.reshape(N, DIN2).T.astype(bf16))  # (192, 8192)
    wpk = _pack_consts(
        W1a, W2a, W1b, W2b, b1a, b2a, b1b, b2b,
        _cnt_inv(r_self), None if share_tgt else _cnt_inv(r_tgt),
    )
    common = {
        "wpk": wpk,
        "bms": _band_mask(r_self).astype(bf16),
    }
    if not share_tgt:
        common["bmt"] = _band_mask(r_tgt).astype(bf16)

    in_maps = []
    for c in range(NCORES):
        x1t = np.ascontiguousarray(
            f1[c * NB : (c + 1) * NB].reshape(NLOC, DIN1).T.astype(bf16)
        )  # (256, 1024)
        # rotate feature2^T columns so this core's rows come first
        x2t = np.ascontiguousarray(
            np.concatenate(
                [x2t_full[:, c * NLOC :], x2t_full[:, : c * NLOC]], axis=1
            )
        )
        in_maps.append({**common, "x1t": x1t, "x2t": x2t})

    res = run_bass_kernel_spmd(
        nc,
        in_maps,
        core_ids=list(range(NCORES)),
        trace=trace,
        **(trace_kwargs or {}),
    )

    pos = np.empty(N, dtype=np.float64)
    se = np.empty(N, dtype=np.float64)
    for c in range(NCORES):
        # column t holds local rows [t*128, (t+1)*128) in partitions
        p = res.results[c]["pos_out"]  # (128, NT)
        s = res.results[c]["se_out"]
        pos[c * NLOC : (c + 1) * NLOC] = p.T.reshape(NLOC)
        se[c * NLOC : (c + 1) * NLOC] = s.T.reshape(NLOC)

    neg = np.log(se) - np.log(float(N))
    loss = np.mean(-pos + neg)
    return np.array(loss, dtype=np.float32), res


# revision 12
# speedup vs baseline: 1.0132x; 1.0132x over previous
"""Contrastive-learning loss kernel for Trainium2 (8 NeuronCores, Bass/Tile).

Problem (hardcoded shapes): B=16, L=512, DIN1=256, DIN2=192, DH=256, DF=128.
  emb1 = MLP_a(feature1); emb2 = MLP_b(feature2)          # (B, L, DF)
  positive = rowdot(f1, f2) + band-mean terms              # (N,)  N = B*L = 8192
  negative = logsumexp(f1 @ f2.T, axis=-1) - log N         # (N,)
  loss = mean(-positive + negative)

Sharding: data-parallel over B for embeddings/positives (2 batches per core);
the N x N negatives matrix is sharded row-wise. Each core computes the full
emb2 from a column-ROTATED copy of feature2 (its own batches first), so the
device program is identical across cores (pure SPMD, no partition-id): the
local rows are always columns [0, 1024) and logsumexp is invariant to column
order.

Schedule notes:
  - ScalarE exp throughput (~64 x 1.25us) is the critical path; everything
    else hides under it.
  - dma_start issue on the Sync queue costs ~1.4us each, so all weights,
    biases and 1/count tables ship as ONE packed bf16 tensor (f32 fields are
    bitcast views); x1t is one DMA; x2 is four. Band masks load after x2.
  - The chain to the first exp is minimal: MLP1 chunk 0 -> MLP2 chunks 0/1
    -> negative matmuls tile 0. Startup relu/bias run on the (otherwise
    idle) ScalarE in parallel with VectorE. MLP1 chunk 1 hides between the
    first negative tiles; transposes/bands/positives spread across the
    8-group loop where PE/DVE have slack.

Outputs per core: pos_out (128, 8), se_out (128, 8) where column t holds
local rows [t*128, (t+1)*128). Host: loss = mean(-pos + log(se) - log N).
"""

import numpy as np

import concourse.bacc as bacc
import concourse.tile as tile
from concourse import mybir
from concourse.bass_utils import run_bass_kernel_spmd
from concourse.masks import make_identity

F32 = mybir.dt.float32
F32R = mybir.dt.float32r
BF16 = mybir.dt.bfloat16

B, L, DIN1, DIN2, DH, DF = 16, 512, 256, 192, 256, 128
N = B * L            # 8192 total rows
NCORES = 8
NB = B // NCORES     # 2 local batches per core
NLOC = NB * L        # 1024 local rows per core
NT = NLOC // 128     # 8 local row tiles
NEG_FD = 1024        # columns exp'd per activation instruction
NGRP = N // NEG_FD   # 8 column groups

# packed-constants column offsets (bf16 columns)
OW1A, OW2A, OW2B, OW1BA, OW1BB = 0, 512, 768, 1024, 1280
OB1A, OB2A, OB1B, OB2B, OCIS, OCIT = 1536, 1540, 1542, 1546, 1548, 1564
CW = 1580


def _build(share_tgt: bool):
    nc = bacc.Bacc("TRN2", target_bir_lowering=False, debug=False)

    x1t_d = nc.dram_tensor("x1t", [DIN1, NLOC], BF16, kind="ExternalInput")
    x2t_d = nc.dram_tensor("x2t", [DIN2, N], BF16, kind="ExternalInput")
    wpk_d = nc.dram_tensor("wpk", [128, CW], BF16, kind="ExternalInput")
    bms_d = nc.dram_tensor("bms", [L, L], BF16, kind="ExternalInput")
    if not share_tgt:
        bmt_d = nc.dram_tensor("bmt", [L, L], BF16, kind="ExternalInput")
    pos_d = nc.dram_tensor("pos_out", [128, NT], F32, kind="ExternalOutput")
    se_d = nc.dram_tensor("se_out", [128, NT], F32, kind="ExternalOutput")

    with tile.TileContext(nc) as tc:
        import contextlib

        with contextlib.ExitStack() as stack:
            const = stack.enter_context(tc.tile_pool(name="const", bufs=1))
            big = stack.enter_context(tc.tile_pool(name="big", bufs=1))
            h2pool = stack.enter_context(tc.tile_pool(name="h2pool", bufs=3))
            posp = stack.enter_context(tc.tile_pool(name="posp", bufs=2))

            # ---- inputs: few, large DMAs, critical-path first. Tiny
            # priority pieces feed the micro-MLP1 -> MLP2-chunk-0 -> first-exp
            # chain while the bulk streams behind them.
            x1t = big.tile([128, 2, NLOC], BF16)
            x1r = x1t_d.ap().rearrange("(t p) c -> p t c", p=128)
            nc.sync.dma_start(out=x1t[:, :, 0:128], in_=x1r[:, :, 0:128])
            wpk = const.tile([128, CW], BF16)
            nc.sync.dma_start(out=wpk[:], in_=wpk_d.ap())
            x2a = big.tile([128, N], BF16)
            x2b = big.tile([64, N], BF16)
            nc.sync.dma_start(out=x2a[:, 0:512], in_=x2t_d.ap()[0:128, 0:512])
            nc.sync.dma_start(out=x2b[:, 0:512], in_=x2t_d.ap()[128:DIN2, 0:512])
            nc.sync.dma_start(out=x1t[:, :, 128:NLOC], in_=x1r[:, :, 128:NLOC])
            nc.sync.dma_start(out=x2a[:, 512:1024], in_=x2t_d.ap()[0:128, 512:1024])
            nc.sync.dma_start(out=x2b[:, 512:1024], in_=x2t_d.ap()[128:DIN2, 512:1024])
            nc.sync.dma_start(out=x2a[:, 1024:4096], in_=x2t_d.ap()[0:128, 1024:4096])
            nc.sync.dma_start(out=x2b[:, 1024:4096], in_=x2t_d.ap()[128:DIN2, 1024:4096])
            nc.sync.dma_start(out=x2a[:, 4096:N], in_=x2t_d.ap()[0:128, 4096:N])
            nc.sync.dma_start(out=x2b[:, 4096:N], in_=x2t_d.ap()[128:DIN2, 4096:N])
            bms = const.tile([128, 4, L], BF16)
            nc.sync.dma_start(
                out=bms[:], in_=bms_d.ap().rearrange("(t p) j -> p t j", p=128)
            )
            if share_tgt:
                bmt = bms
            else:
                bmt = const.tile([128, 4, L], BF16)
                nc.sync.dma_start(
                    out=bmt[:], in_=bmt_d.ap().rearrange("(t p) j -> p t j", p=128)
                )

            def b1a(mt):
                return wpk[:, OB1A + 2 * mt : OB1A + 2 * mt + 2].bitcast(F32)

            def b1b(mt):
                return wpk[:, OB1B + 2 * mt : OB1B + 2 * mt + 2].bitcast(F32)

            b2a = wpk[:, OB2A : OB2A + 2].bitcast(F32)
            b2b = wpk[:, OB2B : OB2B + 2].bitcast(F32)
            cis = wpk[:, OCIS : OCIS + 16].bitcast(F32)
            cit = cis if share_tgt else wpk[:, OCIT : OCIT + 16].bitcast(F32)

            ident = const.tile([128, 128], F32)
            make_identity(nc, ident[:])
            ident_bf = const.tile([128, 128], BF16)
            nc.vector.tensor_copy(ident_bf[:], ident[:])

            e1t = big.tile([128, NLOC], BF16)
            e2t = big.tile([128, N], BF16)
            h1t = big.tile([128, 2, NLOC], BF16)
            e1nat = big.tile([128, NT, DF], BF16)
            e2nat = big.tile([128, NT, DF], BF16)
            w1nat = big.tile([128, NT, DF], F32)
            w2snat = big.tile([128, NT, DF], F32)
            w2tnat = w2snat if share_tgt else big.tile([128, NT, DF], F32)
            pos_all = big.tile([128, NT], F32)
            acc_all = big.tile([128, NT * NGRP + 1], F32)
            se_all = big.tile([128, NT], F32)

            psA = stack.enter_context(tc.tile_pool(name="psumA", bufs=1, space="PSUM"))

            # ---- dummy exp to prefetch the activation table during DMA
            scr = const.tile([128, 8], F32)
            nc.scalar.activation(
                out=scr[:],
                in_=wpk[:, 0:8],
                func=mybir.ActivationFunctionType.Exp,
            )

            def relu_bias(dst, src, bias, use_scalar):
                if use_scalar:
                    nc.scalar.activation(
                        out=dst,
                        in_=src,
                        func=mybir.ActivationFunctionType.Relu,
                        bias=bias,
                    )
                else:
                    nc.vector.tensor_scalar(
                        out=dst,
                        in0=src,
                        scalar1=bias,
                        scalar2=0.0,
                        op0=mybir.AluOpType.add,
                        op1=mybir.AluOpType.max,
                    )

            def add_bias(dst, src, bias, use_scalar):
                if use_scalar:
                    nc.scalar.activation(
                        out=dst,
                        in_=src,
                        func=mybir.ActivationFunctionType.Identity,
                        bias=bias,
                    )
                else:
                    nc.vector.tensor_scalar_add(out=dst, in0=src, scalar1=bias)

            # ---- MLP1: h1T = relu(W1a^T @ x1T + b1a); e1T = W2a^T @ h1T + b2a
            def mlp1_h(cols, sa=False):
                w = cols.stop - cols.start
                h1ps_full = psA.tile([128, 2, 512], F32, tag="hps", bufs=1,
                                     name=f"h1ps{cols.start}")
                h1ps = h1ps_full[:, :, 0:w]
                for mt in range(2):
                    for kt in range(2):
                        nc.tensor.matmul(
                            h1ps[:, mt, :],
                            wpk[:, OW1A + kt * 256 + mt * 128 : OW1A + kt * 256 + (mt + 1) * 128],
                            x1t[:, kt, cols],
                            start=(kt == 0),
                            stop=(kt == 1),
                        )
                for mt in range(2):
                    relu_bias(h1t[:, mt, cols], h1ps[:, mt, :], b1a(mt), sa and mt == 0)

            def mlp1_e(cols, sa=False):
                w = cols.stop - cols.start
                e1ps_full = psA.tile([128, 512], F32, tag="sps", bufs=2,
                                     name=f"e1ps{cols.start}")
                e1ps = e1ps_full[:, 0:w]
                for kt in range(2):
                    nc.tensor.matmul(
                        e1ps[:],
                        wpk[:, OW2A + kt * 128 : OW2A + (kt + 1) * 128],
                        h1t[:, kt, cols],
                        start=(kt == 0),
                        stop=(kt == 1),
                    )
                add_bias(e1t[:, cols], e1ps[:], b2a, sa)

            def transpose_to(dst, srcT, t):
                tp = psA.tile([128, 128], BF16, tag="sps", bufs=2, name=f"tp{t}")
                nc.tensor.transpose(
                    tp[:], srcT[:, t * 128 : (t + 1) * 128], ident_bf[:]
                )
                nc.vector.tensor_copy(dst[:, t, :], tp[:])

            # banded sums: W_sum[j,:] = sum_{|m-j|<=r} e[m,:]  (bf16 matmuls)
            def band_half(dst, bm, src, half):
                for b in range(NB):
                    for jt in (2 * half, 2 * half + 1):
                        wps = psA.tile([128, 128], F32, tag="sps", bufs=2)
                        for mt in range(4):
                            nc.tensor.matmul(
                                wps[:],
                                bm[:, mt, jt * 128 : (jt + 1) * 128],
                                src[:, 4 * b + mt, :],
                                start=(mt == 0),
                                stop=(mt == 3),
                            )
                        nc.vector.tensor_copy(dst[:, 4 * b + jt, :], wps[:])

            # ---- MLP2 over all N tokens ----
            def mlp2_chunk(ct, sa=False):
                cols = slice(ct * 512, (ct + 1) * 512)
                h2ps = psA.tile([128, 2, 512], F32, tag="hps", bufs=1, name=f"h2ps{ct}")
                for mt in range(2):
                    nc.tensor.matmul(
                        h2ps[:, mt, :],
                        wpk[:, OW1BA + mt * 128 : OW1BA + (mt + 1) * 128],
                        x2a[:, cols],
                        start=True,
                        stop=False,
                    )
                    nc.tensor.matmul(
                        h2ps[:, mt, :],
                        wpk[0:64, OW1BB + mt * 128 : OW1BB + (mt + 1) * 128],
                        x2b[:, cols],
                        start=False,
                        stop=True,
                    )
                h2t = h2pool.tile([128, 2, 512], BF16, tag="h2t", name=f"h2t{ct}")
                for mt in range(2):
                    relu_bias(h2t[:, mt, :], h2ps[:, mt, :], b1b(mt), sa and mt == 0)
                e2ps = psA.tile([128, 512], F32, tag="sps", bufs=2, name=f"e2ps{ct}")
                for kt in range(2):
                    nc.tensor.matmul(
                        e2ps[:],
                        wpk[:, OW2B + kt * 128 : OW2B + (kt + 1) * 128],
                        h2t[:, kt, :],
                        start=(kt == 0),
                        stop=(kt == 1),
                    )
                add_bias(e2t[:, cols], e2ps[:], b2b, sa)

            def neg_half(g, t, h, acc_col):
                lhs = e1t[:, t * 128 : (t + 1) * 128]
                np_full = psA.tile([128, NEG_FD], F32, tag="neg", bufs=2, name="nph")
                np_ps = np_full[:, 0:512]
                c0 = g * NEG_FD + h * 512
                nc.tensor.matmul(
                    np_ps[:], lhs, e2t[:, c0 : c0 + 512], start=True, stop=True
                )
                nc.scalar.activation(
                    out=np_ps[:],
                    in_=np_ps[:],
                    func=mybir.ActivationFunctionType.Exp,
                    accum_out=acc_all[:, acc_col : acc_col + 1],
                )

            def neg_tile(g, t):
                lhs = e1t[:, t * 128 : (t + 1) * 128]
                np_ps = psA.tile([128, NEG_FD], F32, tag="neg", bufs=2)
                for i in range(NEG_FD // 512):
                    c0 = g * NEG_FD + i * 512
                    nc.tensor.matmul(
                        np_ps[:, i * 512 : (i + 1) * 512],
                        lhs,
                        e2t[:, c0 : c0 + 512],
                        start=True,
                        stop=True,
                    )
                idx = t * NGRP + g
                nc.scalar.activation(
                    out=np_ps[:],
                    in_=np_ps[:],
                    func=mybir.ActivationFunctionType.Exp,
                    accum_out=acc_all[:, idx : idx + 1],
                )

            # positives for one local batch b (VectorE only)
            def positives(b):
                bsl = slice(4 * b, 4 * b + 4)
                ga = posp.tile([128, 4, DF], F32, tag="posg")
                r1 = posp.tile([128, 4], F32, tag="post")
                r2 = posp.tile([128, 4], F32, tag="post")
                if share_tgt:
                    nc.vector.tensor_add(ga[:], w1nat[:, bsl, :], w2snat[:, bsl, :])
                    nc.vector.tensor_mul(ga[:], ga[:], e1nat[:, bsl, :])
                else:
                    nc.vector.tensor_mul(ga[:], w1nat[:, bsl, :], e1nat[:, bsl, :])
                nc.vector.tensor_reduce(
                    out=r1[:], in_=ga[:], axis=mybir.AxisListType.X, op=mybir.AluOpType.add
                )
                gb = posp.tile([128, 4, DF], F32, tag="posg")
                nc.vector.tensor_mul(gb[:], w2snat[:, bsl, :], e2nat[:, bsl, :])
                nc.vector.tensor_reduce(
                    out=r2[:], in_=gb[:], axis=mybir.AxisListType.X, op=mybir.AluOpType.add
                )
                nc.vector.tensor_add(r1[:], r1[:], r2[:])
                nc.vector.tensor_mul(r1[:], r1[:], cis[:, bsl])
                if not share_tgt:
                    gc = posp.tile([128, 4, DF], F32, tag="posg")
                    nc.vector.tensor_mul(gc[:], w2tnat[:, bsl, :], e1nat[:, bsl, :])
                    rt = posp.tile([128, 4], F32, tag="post")
                    nc.vector.tensor_reduce(
                        out=rt[:], in_=gc[:], axis=mybir.AxisListType.X,
                        op=mybir.AluOpType.add,
                    )
                    nc.vector.tensor_mul(rt[:], rt[:], cit[:, bsl])
                    nc.vector.tensor_add(r1[:], r1[:], rt[:])
                gd = posp.tile([128, 4, DF], BF16, tag="posgb")
                nc.vector.tensor_mul(gd[:], e1nat[:, bsl, :], e2nat[:, bsl, :])
                r3 = posp.tile([128, 4], F32, tag="post")
                nc.vector.tensor_reduce(
                    out=r3[:], in_=gd[:], axis=mybir.AxisListType.X, op=mybir.AluOpType.add
                )
                nc.vector.tensor_add(pos_all[:, bsl], r1[:], r3[:])

            # per-group deferred work under the exp-bound steady state
            if share_tgt:
                extras = {
                    0: [lambda: [transpose_to(e1nat, e1t, t) for t in range(NT)]],
                    1: [lambda: [transpose_to(e2nat, e2t, t) for t in range(NT)]],
                    2: [lambda: band_half(w1nat, bms, e1nat, 0)],
                    3: [lambda: band_half(w1nat, bms, e1nat, 1)],
                    4: [lambda: band_half(w2snat, bms, e2nat, 0)],
                    5: [lambda: band_half(w2snat, bms, e2nat, 1)],
                    6: [lambda: positives(0)],
                    7: [lambda: positives(1)],
                }
            else:
                extras = {
                    0: [lambda: [transpose_to(e1nat, e1t, t) for t in range(NT)]],
                    1: [lambda: [transpose_to(e2nat, e2t, t) for t in range(NT)]],
                    2: [lambda: band_half(w1nat, bms, e1nat, 0),
                        lambda: band_half(w1nat, bms, e1nat, 1)],
                    3: [lambda: band_half(w2snat, bms, e2nat, 0)],
                    4: [lambda: band_half(w2snat, bms, e2nat, 1)],
                    5: [lambda: band_half(w2tnat, bmt, e2nat, 0)],
                    6: [lambda: band_half(w2tnat, bmt, e2nat, 1)],
                    7: [lambda: positives(0), lambda: positives(1)],
                }

            # ---- minimal-latency chain to the first exp: micro-MLP1 for the
            # first 128 e1 columns, MLP2 chunk 0, then an F=512 exp on row
            # tile 0 (its two halves use accum columns 64 and 0). The rest
            # of MLP1 and the deferred work hide under the exp stream.
            mlp1_h(slice(0, 128), sa=True)
            mlp1_e(slice(0, 128), sa=True)
            mlp2_chunk(0, sa=True)
            neg_half(0, 0, 0, NT * NGRP)
            mlp2_chunk(1, sa=True)
            neg_half(0, 0, 1, 0)
            mlp1_h(slice(128, 512))
            mlp1_e(slice(128, 512))
            neg_tile(0, 1)
            neg_tile(0, 2)
            mlp1_h(slice(512, 1024))
            neg_tile(0, 3)
            mlp1_e(slice(512, 1024))
            neg_tile(0, 4)
            neg_tile(0, 5)
            mlp2_chunk(2)
            neg_tile(0, 6)
            neg_tile(0, 7)
            mlp2_chunk(3)
            for fn in extras[0]:
                fn()
            for g in range(1, NGRP):
                for t in range(4):
                    neg_tile(g, t)
                if g < NGRP - 1:
                    mlp2_chunk(2 * g + 2)
                neg_tile(g, 4)
                neg_tile(g, 5)
                if g < NGRP - 1:
                    mlp2_chunk(2 * g + 3)
                neg_tile(g, 6)
                neg_tile(g, 7)
                for fn in extras[g]:
                    fn()

            nc.sync.dma_start(out=pos_d.ap(), in_=pos_all[:])
            for t in range(NT):
                nc.vector.tensor_reduce(
                    out=se_all[:, t : t + 1],
                    in_=acc_all[:, t * NGRP : (t + 1) * NGRP],
                    axis=mybir.AxisListType.X,
                    op=mybir.AluOpType.add,
                )
            nc.vector.tensor_add(
                se_all[:, 0:1], se_all[:, 0:1], acc_all[:, NT * NGRP : NT * NGRP + 1]
            )
            nc.sync.dma_start(out=se_d.ap(), in_=se_all[:])

    nc.compile()
    return nc


_BUILD_CACHE: dict = {}


def _get_nc(share_tgt: bool):
    if share_tgt not in _BUILD_CACHE:
        _BUILD_CACHE[share_tgt] = _build(share_tgt)
    return _BUILD_CACHE[share_tgt]


def _band_mask(r: int) -> np.ndarray:
    """mask[m, j] = 1 if |m-j| <= r (and inside [0,L)) else 0."""
    bm = np.zeros((L, L), dtype=np.float32)
    if r > 0:
        j = np.arange(L)
        lo = np.maximum(j - r, 0)
        hi = np.minimum(j + r + 1, L)
        m = np.arange(L)[:, None]
        bm = ((m >= lo[None, :]) & (m < hi[None, :])).astype(np.float32)
    return bm


def _cnt_inv(r: int) -> np.ndarray:
    """(128, NT) tile of 1/count(j) per local row (j = row mod L)."""
    j = np.arange(L)
    if r > 0:
        cnt = (np.minimum(j + r + 1, L) - np.maximum(j - r, 0)).astype(np.float64)
    else:
        cnt = np.ones(L)
    cinv = (1.0 / cnt).astype(np.float32)
    rows = (np.arange(NLOC) % L)
    return np.ascontiguousarray(cinv[rows].reshape(NT, 128).T)


def _pack_consts(W1a, W2a, W1b, W2b, b1a, b2a, b1b, b2b, cis, cit):
    """One (128, CW) bf16 tensor holding every weight/bias/count table.
    f32 fields are stored as raw byte pairs (device reads them via bitcast)."""
    import ml_dtypes

    bf16 = ml_dtypes.bfloat16
    pk = np.zeros((128, CW), dtype=np.uint16)

    def put_bf(col, arr):
        a = np.ascontiguousarray(arr.astype(bf16)).view(np.uint16)
        pk[: a.shape[0], col : col + a.shape[1]] = a

    def put_f32(col, arr):
        a = np.ascontiguousarray(arr.astype(np.float32)).view(np.uint16)
        pk[: a.shape[0], col : col + a.shape[1]] = a

    # w1a[p, kt*256 + m] = W1a[kt*128 + p, m]
    put_bf(OW1A, np.asarray(W1a, np.float32).reshape(2, 128, DH).transpose(1, 0, 2).reshape(128, 512))
    put_bf(OW2A, np.asarray(W2a, np.float32).reshape(2, 128, DF).transpose(1, 0, 2).reshape(128, 256))
    put_bf(OW2B, np.asarray(W2b, np.float32).reshape(2, 128, DF).transpose(1, 0, 2).reshape(128, 256))
    w1b = np.asarray(W1b, np.float32)
    put_bf(OW1BA, w1b[0:128, :])
    # w1b_b for mt: partitions 0:64 of cols OW1BB+128*mt = W1b[128+r, 128*mt+c]
    put_bf(OW1BB, w1b[128:192, :].reshape(64, 2, 128).transpose(0, 1, 2).reshape(64, 256))
    put_f32(OB1A, np.asarray(b1a, np.float32).reshape(2, 128).T)
    put_f32(OB2A, np.asarray(b2a, np.float32).reshape(128, 1))
    put_f32(OB1B, np.asarray(b1b, np.float32).reshape(2, 128).T)
    put_f32(OB2B, np.asarray(b2b, np.float32).reshape(128, 1))
    put_f32(OCIS, cis)
    if cit is not None:
        put_f32(OCIT, cit)
    return pk.view(bf16)


def kernel(**inputs):
    loss, _ = _run(inputs, trace=False)
    return loss


def _run(inputs, trace=False, trace_kwargs=None):
    import ml_dtypes

    bf16 = ml_dtypes.bfloat16
    feature1 = inputs["feature1"]
    feature2 = inputs["feature2"]
    W1a, b1a, W2a, b2a = inputs["W1a"], inputs["b1a"], inputs["W2a"], inputs["b2a"]
    W1b, b1b, W2b, b2b = inputs["W1b"], inputs["b1b"], inputs["W2b"], inputs["b2b"]
    f1 = np.ascontiguousarray(np.asarray(feature1, dtype=np.float32))
    f2 = np.ascontiguousarray(np.asarray(feature2, dtype=np.float32))
    r_self = int(np.asarray(inputs["positive_range_self"]))
    r_tgt = int(np.asarray(inputs["positive_range_tgt"]))
    share_tgt = r_tgt == r_self

    nc = _get_nc(share_tgt)

    x2t_full = np.ascontiguousarray(f2.reshape(N, DIN2).T.astype(bf16))  # (192, 8192)
    wpk = _pack_consts(
        W1a, W2a, W1b, W2b, b1a, b2a, b1b, b2b,
        _cnt_inv(r_self), None if share_tgt else _cnt_inv(r_tgt),
    )
    common = {
        "wpk": wpk,
        "bms": _band_mask(r_self).astype(bf16),
    }
    if not share_tgt:
        common["bmt"] = _band_mask(r_tgt).astype(bf16)

    in_maps = []
    for c in range(NCORES):
        x1t = np.ascontiguousarray(
            f1[c * NB : (c + 1) * NB].reshape(NLOC, DIN1).T.astype(bf16)
        )  # (256, 1024)
        # rotate feature2^T columns so this core's rows come first
        x2t = np.ascontiguousarray(
            np.concatenate(
                [x2t_full[:, c * NLOC :], x2t_full[:, : c * NLOC]], axis=1
            )
        )
        in_maps.append({**common, "x1t": x1t, "x2t": x2t})

    res = run_bass_kernel_spmd(
        nc,
        in_maps,
        core_ids=list(range(NCORES)),
        trace=trace,
        **(trace_kwargs or {}),
    )

    pos = np.empty(N, dtype=np.float64)
    se = np.empty(N, dtype=np.float64)
    for c in range(NCORES):
        # column t holds local rows [t*128, (t+1)*128) in partitions
        p = res.results[c]["pos_out"]  # (128, NT)
        s = res.results[c]["se_out"]
        pos[c * NLOC : (c + 1) * NLOC] = p.T.reshape(NLOC)
        se[c * NLOC : (c + 1) * NLOC] = s.T.reshape(NLOC)

    neg = np.log(se) - np.log(float(N))
    loss = np.mean(-pos + neg)
    return np.array(loss, dtype=np.float32), res


# revision 13
# speedup vs baseline: 1.0235x; 1.0102x over previous
"""Contrastive-learning loss kernel for Trainium2 (8 NeuronCores, Bass/Tile).

Problem (hardcoded shapes): B=16, L=512, DIN1=256, DIN2=192, DH=256, DF=128.
  emb1 = MLP_a(feature1); emb2 = MLP_b(feature2)          # (B, L, DF)
  positive = rowdot(f1, f2) + band-mean terms              # (N,)  N = B*L = 8192
  negative = logsumexp(f1 @ f2.T, axis=-1) - log N         # (N,)
  loss = mean(-positive + negative)

Sharding: data-parallel over B for embeddings/positives (2 batches per core);
the N x N negatives matrix is sharded row-wise. Each core computes the full
emb2 from a column-ROTATED copy of feature2 (its own batches first), so the
device program is identical across cores (pure SPMD, no partition-id): the
local rows are always columns [0, 1024) and logsumexp is invariant to column
order.

Schedule notes:
  - ScalarE exp throughput (~64 x 1.25us) is the critical path; everything
    else hides under it.
  - dma_start issue on the Sync queue costs ~1.4us each, so all weights,
    biases and 1/count tables ship as ONE packed bf16 tensor (f32 fields are
    bitcast views); x1t is one DMA; x2 is four. Band masks load after x2.
  - The chain to the first exp is minimal: MLP1 chunk 0 -> MLP2 chunks 0/1
    -> negative matmuls tile 0. Startup relu/bias run on the (otherwise
    idle) ScalarE in parallel with VectorE. MLP1 chunk 1 hides between the
    first negative tiles; transposes/bands/positives spread across the
    8-group loop where PE/DVE have slack.

Outputs per core: pos_out (128, 8), se_out (128, 8) where column t holds
local rows [t*128, (t+1)*128). Host: loss = mean(-pos + log(se) - log N).
"""

import numpy as np

import concourse.bacc as bacc
import concourse.tile as tile
from concourse import mybir
from concourse.bass_utils import run_bass_kernel_spmd
from concourse.masks import make_identity

F32 = mybir.dt.float32
F32R = mybir.dt.float32r
BF16 = mybir.dt.bfloat16

B, L, DIN1, DIN2, DH, DF = 16, 512, 256, 192, 256, 128
N = B * L            # 8192 total rows
NCORES = 8
NB = B // NCORES     # 2 local batches per core
NLOC = NB * L        # 1024 local rows per core
NT = NLOC // 128     # 8 local row tiles
NEG_FD = 1024        # columns exp'd per activation instruction
NGRP = N // NEG_FD   # 8 column groups

# packed-constants column offsets (bf16 columns)
OW1A, OW2A, OW2B, OW1BA, OW1BB = 0, 512, 768, 1024, 1280
OB1A, OB2A, OB1B, OB2B, OCIS, OCIT = 1536, 1540, 1542, 1546, 1548, 1564
CW = 1580


def _build(share_tgt: bool):
    nc = bacc.Bacc("TRN2", target_bir_lowering=False, debug=False)

    x1t_d = nc.dram_tensor("x1t", [DIN1, NLOC], BF16, kind="ExternalInput")
    x2t_d = nc.dram_tensor("x2t", [DIN2, N], BF16, kind="ExternalInput")
    wpk_d = nc.dram_tensor("wpk", [128, CW], BF16, kind="ExternalInput")
    bms_d = nc.dram_tensor("bms", [L, L], BF16, kind="ExternalInput")
    if not share_tgt:
        bmt_d = nc.dram_tensor("bmt", [L, L], BF16, kind="ExternalInput")
    pos_d = nc.dram_tensor("pos_out", [128, NT], F32, kind="ExternalOutput")
    se_d = nc.dram_tensor("se_out", [128, NT], F32, kind="ExternalOutput")

    with tile.TileContext(nc) as tc:
        import contextlib

        with contextlib.ExitStack() as stack:
            const = stack.enter_context(tc.tile_pool(name="const", bufs=1))
            big = stack.enter_context(tc.tile_pool(name="big", bufs=1))
            h2pool = stack.enter_context(tc.tile_pool(name="h2pool", bufs=3))
            posp = stack.enter_context(tc.tile_pool(name="posp", bufs=2))

            # ---- inputs: few, large DMAs, critical-path first. Tiny
            # priority pieces feed the micro-MLP1 -> MLP2-chunk-0 -> first-exp
            # chain while the bulk streams behind them.
            x1t = big.tile([128, 2, NLOC], BF16)
            x1r = x1t_d.ap().rearrange("(t p) c -> p t c", p=128)
            nc.sync.dma_start(out=x1t[:, :, 0:128], in_=x1r[:, :, 0:128])
            wpk = const.tile([128, CW], BF16)
            nc.sync.dma_start(out=wpk[:], in_=wpk_d.ap())
            x2a = big.tile([128, N], BF16)
            x2b = big.tile([64, N], BF16)
            nc.sync.dma_start(out=x2a[:, 0:512], in_=x2t_d.ap()[0:128, 0:512])
            nc.sync.dma_start(out=x2b[:, 0:512], in_=x2t_d.ap()[128:DIN2, 0:512])
            nc.sync.dma_start(out=x1t[:, :, 128:NLOC], in_=x1r[:, :, 128:NLOC])
            nc.sync.dma_start(out=x2a[:, 512:1024], in_=x2t_d.ap()[0:128, 512:1024])
            nc.sync.dma_start(out=x2b[:, 512:1024], in_=x2t_d.ap()[128:DIN2, 512:1024])
            nc.sync.dma_start(out=x2a[:, 1024:4096], in_=x2t_d.ap()[0:128, 1024:4096])
            nc.sync.dma_start(out=x2b[:, 1024:4096], in_=x2t_d.ap()[128:DIN2, 1024:4096])
            nc.sync.dma_start(out=x2a[:, 4096:N], in_=x2t_d.ap()[0:128, 4096:N])
            nc.sync.dma_start(out=x2b[:, 4096:N], in_=x2t_d.ap()[128:DIN2, 4096:N])
            bms = const.tile([128, 4, L], BF16)
            nc.sync.dma_start(
                out=bms[:], in_=bms_d.ap().rearrange("(t p) j -> p t j", p=128)
            )
            if share_tgt:
                bmt = bms
            else:
                bmt = const.tile([128, 4, L], BF16)
                nc.sync.dma_start(
                    out=bmt[:], in_=bmt_d.ap().rearrange("(t p) j -> p t j", p=128)
                )

            def b1a(mt):
                return wpk[:, OB1A + 2 * mt : OB1A + 2 * mt + 2].bitcast(F32)

            def b1b(mt):
                return wpk[:, OB1B + 2 * mt : OB1B + 2 * mt + 2].bitcast(F32)

            b2a = wpk[:, OB2A : OB2A + 2].bitcast(F32)
            b2b = wpk[:, OB2B : OB2B + 2].bitcast(F32)
            cis = wpk[:, OCIS : OCIS + 16].bitcast(F32)
            cit = cis if share_tgt else wpk[:, OCIT : OCIT + 16].bitcast(F32)

            ident = const.tile([128, 128], F32)
            make_identity(nc, ident[:])
            ident_bf = const.tile([128, 128], BF16)
            nc.vector.tensor_copy(ident_bf[:], ident[:])

            e1t = big.tile([128, NLOC], BF16)
            e2t = big.tile([128, N], BF16)
            h1t = big.tile([128, 2, NLOC], BF16)
            e1nat = big.tile([128, NT, DF], BF16)
            e2nat = big.tile([128, NT, DF], BF16)
            w1nat = big.tile([128, NT, DF], F32)
            w2snat = big.tile([128, NT, DF], F32)
            w2tnat = w2snat if share_tgt else big.tile([128, NT, DF], F32)
            pos_all = big.tile([128, NT], F32)
            acc_all = big.tile([128, NT * NGRP + 1], F32)
            se_all = big.tile([128, NT], F32)

            psA = stack.enter_context(tc.tile_pool(name="psumA", bufs=1, space="PSUM"))

            # ---- dummy exp to prefetch the activation table during DMA
            scr = const.tile([128, 8], F32)
            nc.scalar.activation(
                out=scr[:],
                in_=wpk[:, 0:8],
                func=mybir.ActivationFunctionType.Exp,
            )

            def relu_bias(dst, src, bias, use_scalar):
                if use_scalar:
                    nc.scalar.activation(
                        out=dst,
                        in_=src,
                        func=mybir.ActivationFunctionType.Relu,
                        bias=bias,
                    )
                else:
                    nc.vector.tensor_scalar(
                        out=dst,
                        in0=src,
                        scalar1=bias,
                        scalar2=0.0,
                        op0=mybir.AluOpType.add,
                        op1=mybir.AluOpType.max,
                    )

            def add_bias(dst, src, bias, use_scalar):
                if use_scalar:
                    nc.scalar.activation(
                        out=dst,
                        in_=src,
                        func=mybir.ActivationFunctionType.Identity,
                        bias=bias,
                    )
                else:
                    nc.vector.tensor_scalar_add(out=dst, in0=src, scalar1=bias)

            # ---- MLP1: h1T = relu(W1a^T @ x1T + b1a); e1T = W2a^T @ h1T + b2a
            def mlp1_h(cols, sa=False):
                w = cols.stop - cols.start
                h1ps_full = psA.tile([128, 2, 512], F32, tag="hps", bufs=1,
                                     name=f"h1ps{cols.start}")
                h1ps = h1ps_full[:, :, 0:w]
                for mt in range(2):
                    for kt in range(2):
                        nc.tensor.matmul(
                            h1ps[:, mt, :],
                            wpk[:, OW1A + kt * 256 + mt * 128 : OW1A + kt * 256 + (mt + 1) * 128],
                            x1t[:, kt, cols],
                            start=(kt == 0),
                            stop=(kt == 1),
                        )
                for mt in range(2):
                    relu_bias(h1t[:, mt, cols], h1ps[:, mt, :], b1a(mt), sa and mt == 0)

            def mlp1_e(cols, sa=False):
                w = cols.stop - cols.start
                e1ps_full = psA.tile([128, 512], F32, tag="sps", bufs=2,
                                     name=f"e1ps{cols.start}")
                e1ps = e1ps_full[:, 0:w]
                for kt in range(2):
                    nc.tensor.matmul(
                        e1ps[:],
                        wpk[:, OW2A + kt * 128 : OW2A + (kt + 1) * 128],
                        h1t[:, kt, cols],
                        start=(kt == 0),
                        stop=(kt == 1),
                    )
                add_bias(e1t[:, cols], e1ps[:], b2a, sa)

            def transpose_to(dst, srcT, t):
                tp = psA.tile([128, 128], BF16, tag="sps", bufs=2, name=f"tp{t}")
                nc.tensor.transpose(
                    tp[:], srcT[:, t * 128 : (t + 1) * 128], ident_bf[:]
                )
                nc.vector.tensor_copy(dst[:, t, :], tp[:])

            # banded sums: W_sum[j,:] = sum_{|m-j|<=r} e[m,:]  (bf16 matmuls)
            def band_half(dst, bm, src, half):
                for b in range(NB):
                    for jt in (2 * half, 2 * half + 1):
                        wps = psA.tile([128, 128], F32, tag="sps", bufs=2)
                        for mt in range(4):
                            nc.tensor.matmul(
                                wps[:],
                                bm[:, mt, jt * 128 : (jt + 1) * 128],
                                src[:, 4 * b + mt, :],
                                start=(mt == 0),
                                stop=(mt == 3),
                            )
                        nc.vector.tensor_copy(dst[:, 4 * b + jt, :], wps[:])

            # ---- MLP2 over all N tokens ----
            def mlp2_chunk(ct, sa=False):
                cols = slice(ct * 512, (ct + 1) * 512)
                h2ps = psA.tile([128, 2, 512], F32, tag="hps", bufs=1, name=f"h2ps{ct}")
                for mt in range(2):
                    nc.tensor.matmul(
                        h2ps[:, mt, :],
                        wpk[:, OW1BA + mt * 128 : OW1BA + (mt + 1) * 128],
                        x2a[:, cols],
                        start=True,
                        stop=False,
                    )
                    nc.tensor.matmul(
                        h2ps[:, mt, :],
                        wpk[0:64, OW1BB + mt * 128 : OW1BB + (mt + 1) * 128],
                        x2b[:, cols],
                        start=False,
                        stop=True,
                    )
                h2t = h2pool.tile([128, 2, 512], BF16, tag="h2t", name=f"h2t{ct}")
                for mt in range(2):
                    relu_bias(h2t[:, mt, :], h2ps[:, mt, :], b1b(mt), sa and mt == 0)
                e2ps = psA.tile([128, 512], F32, tag="sps", bufs=2, name=f"e2ps{ct}")
                for kt in range(2):
                    nc.tensor.matmul(
                        e2ps[:],
                        wpk[:, OW2B + kt * 128 : OW2B + (kt + 1) * 128],
                        h2t[:, kt, :],
                        start=(kt == 0),
                        stop=(kt == 1),
                    )
                add_bias(e2t[:, cols], e2ps[:], b2b, sa)

            def neg_half(g, t, h, acc_col):
                lhs = e1t[:, t * 128 : (t + 1) * 128]
                np_full = psA.tile([128, NEG_FD], F32, tag="neg", bufs=2, name="nph")
                np_ps = np_full[:, 0:512]
                c0 = g * NEG_FD + h * 512
                nc.tensor.matmul(
                    np_ps[:], lhs, e2t[:, c0 : c0 + 512], start=True, stop=True
                )
                nc.scalar.activation(
                    out=np_ps[:],
                    in_=np_ps[:],
                    func=mybir.ActivationFunctionType.Exp,
                    accum_out=acc_all[:, acc_col : acc_col + 1],
                )

            def neg_tile(g, t):
                lhs = e1t[:, t * 128 : (t + 1) * 128]
                np_ps = psA.tile([128, NEG_FD], F32, tag="neg", bufs=2)
                for i in range(NEG_FD // 512):
                    c0 = g * NEG_FD + i * 512
                    nc.tensor.matmul(
                        np_ps[:, i * 512 : (i + 1) * 512],
                        lhs,
                        e2t[:, c0 : c0 + 512],
                        start=True,
                        stop=True,
                    )
                idx = t * NGRP + g
                nc.scalar.activation(
                    out=np_ps[:],
                    in_=np_ps[:],
                    func=mybir.ActivationFunctionType.Exp,
                    accum_out=acc_all[:, idx : idx + 1],
                )

            # positives for one local batch b (VectorE only)
            def positives(b):
                bsl = slice(4 * b, 4 * b + 4)
                ga = posp.tile([128, 4, DF], F32, tag="posg")
                r1 = posp.tile([128, 4], F32, tag="post")
                r2 = posp.tile([128, 4], F32, tag="post")
                if share_tgt:
                    nc.vector.tensor_add(ga[:], w1nat[:, bsl, :], w2snat[:, bsl, :])
                    nc.vector.tensor_mul(ga[:], ga[:], e1nat[:, bsl, :])
                else:
                    nc.vector.tensor_mul(ga[:], w1nat[:, bsl, :], e1nat[:, bsl, :])
                nc.vector.tensor_reduce(
                    out=r1[:], in_=ga[:], axis=mybir.AxisListType.X, op=mybir.AluOpType.add
                )
                gb = posp.tile([128, 4, DF], F32, tag="posg")
                nc.vector.tensor_mul(gb[:], w2snat[:, bsl, :], e2nat[:, bsl, :])
                nc.vector.tensor_reduce(
                    out=r2[:], in_=gb[:], axis=mybir.AxisListType.X, op=mybir.AluOpType.add
                )
                nc.vector.tensor_add(r1[:], r1[:], r2[:])
                nc.vector.tensor_mul(r1[:], r1[:], cis[:, bsl])
                if not share_tgt:
                    gc = posp.tile([128, 4, DF], F32, tag="posg")
                    nc.vector.tensor_mul(gc[:], w2tnat[:, bsl, :], e1nat[:, bsl, :])
                    rt = posp.tile([128, 4], F32, tag="post")
                    nc.vector.tensor_reduce(
                        out=rt[:], in_=gc[:], axis=mybir.AxisListType.X,
                        op=mybir.AluOpType.add,
                    )
                    nc.vector.tensor_mul(rt[:], rt[:], cit[:, bsl])
                    nc.vector.tensor_add(r1[:], r1[:], rt[:])
                gd = posp.tile([128, 4, DF], BF16, tag="posgb")
                nc.vector.tensor_mul(gd[:], e1nat[:, bsl, :], e2nat[:, bsl, :])
                r3 = posp.tile([128, 4], F32, tag="post")
                nc.vector.tensor_reduce(
                    out=r3[:], in_=gd[:], axis=mybir.AxisListType.X, op=mybir.AluOpType.add
                )
                nc.vector.tensor_add(pos_all[:, bsl], r1[:], r3[:])

            # per-group deferred work under the exp-bound steady state
            if share_tgt:
                extras = {
                    0: [],
                    1: [lambda: [transpose_to(e1nat, e1t, t) for t in range(NT)]],
                    2: [lambda: [transpose_to(e2nat, e2t, t) for t in range(NT)]],
                    3: [lambda: band_half(w1nat, bms, e1nat, 0)],
                    4: [lambda: band_half(w1nat, bms, e1nat, 1)],
                    5: [lambda: band_half(w2snat, bms, e2nat, 0)],
                    6: [lambda: band_half(w2snat, bms, e2nat, 1)],
                    7: [lambda: positives(0), lambda: positives(1)],
                }
            else:
                extras = {
                    0: [],
                    1: [lambda: [transpose_to(e1nat, e1t, t) for t in range(NT)]],
                    2: [lambda: [transpose_to(e2nat, e2t, t) for t in range(NT)]],
                    3: [lambda: band_half(w1nat, bms, e1nat, 0),
                        lambda: band_half(w1nat, bms, e1nat, 1)],
                    4: [lambda: band_half(w2snat, bms, e2nat, 0)],
                    5: [lambda: band_half(w2snat, bms, e2nat, 1)],
                    6: [lambda: band_half(w2tnat, bmt, e2nat, 0)],
                    7: [lambda: band_half(w2tnat, bmt, e2nat, 1),
                        lambda: positives(0), lambda: positives(1)],
                }

            # ---- minimal-latency chain to the first exp: micro-MLP1 for the
            # first 128 e1 columns, MLP2 chunk 0, then an F=512 exp on row
            # tile 0 (its two halves use accum columns 64 and 0). The rest
            # of MLP1 and the deferred work hide under the exp stream.
            mlp1_h(slice(0, 128), sa=True)
            mlp1_e(slice(0, 128), sa=True)
            mlp2_chunk(0, sa=True)
            neg_half(0, 0, 0, NT * NGRP)
            mlp2_chunk(1, sa=True)
            neg_half(0, 0, 1, 0)
            mlp1_h(slice(128, 512))
            mlp1_e(slice(128, 512))
            neg_tile(0, 1)
            neg_tile(0, 2)
            mlp1_h(slice(512, 1024))
            neg_tile(0, 3)
            mlp1_e(slice(512, 1024))
            neg_tile(0, 4)
            neg_tile(0, 5)
            mlp2_chunk(2)
            neg_tile(0, 6)
            neg_tile(0, 7)
            mlp2_chunk(3)
            for fn in extras[0]:
                fn()
            for g in range(1, NGRP):
                for t in range(4):
                    neg_tile(g, t)
                if g < NGRP - 1:
                    mlp2_chunk(2 * g + 2)
                neg_tile(g, 4)
                neg_tile(g, 5)
                if g < NGRP - 1:
                    mlp2_chunk(2 * g + 3)
                neg_tile(g, 6)
                neg_tile(g, 7)
                for fn in extras[g]:
                    fn()

            nc.sync.dma_start(out=pos_d.ap(), in_=pos_all[:])
            for t in range(NT):
                nc.vector.tensor_reduce(
                    out=se_all[:, t : t + 1],
                    in_=acc_all[:, t * NGRP : (t + 1) * NGRP],
                    axis=mybir.AxisListType.X,
                    op=mybir.AluOpType.add,
                )
            nc.vector.tensor_add(
                se_all[:, 0:1], se_all[:, 0:1], acc_all[:, NT * NGRP : NT * NGRP + 1]
            )
            nc.sync.dma_start(out=se_d.ap(), in_=se_all[:])

    nc.compile()
    return nc


_BUILD_CACHE: dict = {}


def _get_nc(share_tgt: bool):
    if share_tgt not in _BUILD_CACHE:
        _BUILD_CACHE[share_tgt] = _build(share_tgt)
    return _BUILD_CACHE[share_tgt]


def _band_mask(r: int) -> np.ndarray:
    """mask[m, j] = 1 if |m-j| <= r (and inside [0,L)) else 0."""
    bm = np.zeros((L, L), dtype=np.float32)
    if r > 0:
        j = np.arange(L)
        lo = np.maximum(j - r, 0)
        hi = np.minimum(j + r + 1, L)
        m = np.arange(L)[:, None]
        bm = ((m >= lo[None, :]) & (m < hi[None, :])).astype(np.float32)
    return bm


def _cnt_inv(r: int) -> np.ndarray:
    """(128, NT) tile of 1/count(j) per local row (j = row mod L)."""
    j = np.arange(L)
    if r > 0:
        cnt = (np.minimum(j + r + 1, L) - np.maximum(j - r, 0)).astype(np.float64)
    else:
        cnt = np.ones(L)
    cinv = (1.0 / cnt).astype(np.float32)
    rows = (np.arange(NLOC) % L)
    return np.ascontiguousarray(cinv[rows].reshape(NT, 128).T)


def _pack_consts(W1a, W2a, W1b, W2b, b1a, b2a, b1b, b2b, cis, cit):
    """One (128, CW) bf16 tensor holding every weight/bias/count table.
    f32 fields are stored as raw byte pairs (device reads them via bitcast)."""
    import ml_dtypes

    bf16 = ml_dtypes.bfloat16
    pk = np.zeros((128, CW), dtype=np.uint16)

    def put_bf(col, arr):
        a = np.ascontiguousarray(arr.astype(bf16)).view(np.uint16)
        pk[: a.shape[0], col : col + a.shape[1]] = a

    def put_f32(col, arr):
        a = np.ascontiguousarray(arr.astype(np.float32)).view(np.uint16)
        pk[: a.shape[0], col : col + a.shape[1]] = a

    # w1a[p, kt*256 + m] = W1a[kt*128 + p, m]
    put_bf(OW1A, np.asarray(W1a, np.float32).reshape(2, 128, DH).transpose(1, 0, 2).reshape(128, 512))
    put_bf(OW2A, np.asarray(W2a, np.float32).reshape(2, 128, DF).transpose(1, 0, 2).reshape(128, 256))
    put_bf(OW2B, np.asarray(W2b, np.float32).reshape(2, 128, DF).transpose(1, 0, 2).reshape(128, 256))
    w1b = np.asarray(W1b, np.float32)
    put_bf(OW1BA, w1b[0:128, :])
    # w1b_b for mt: partitions 0:64 of cols OW1BB+128*mt = W1b[128+r, 128*mt+c]
    put_bf(OW1BB, w1b[128:192, :].reshape(64, 2, 128).transpose(0, 1, 2).reshape(64, 256))
    put_f32(OB1A, np.asarray(b1a, np.float32).reshape(2, 128).T)
    put_f32(OB2A, np.asarray(b2a, np.float32).reshape(128, 1))
    put_f32(OB1B, np.asarray(b1b, np.float32).reshape(2, 128).T)
    put_f32(OB2B, np.asarray(b2b, np.float32).reshape(128, 1))
    put_f32(OCIS, cis)
    if cit is not None:
        put_f32(OCIT, cit)
    return pk.view(bf16)


def kernel(**inputs):
    loss, _ = _run(inputs, trace=False)
    return loss


def _run(inputs, trace=False, trace_kwargs=None):
    import ml_dtypes

    bf16 = ml_dtypes.bfloat16
    feature1 = inputs["feature1"]
    feature2 = inputs["feature2"]
    W1a, b1a, W2a, b2a = inputs["W1a"], inputs["b1a"], inputs["W2a"], inputs["b2a"]
    W1b, b1b, W2b, b2b = inputs["W1b"], inputs["b1b"], inputs["W2b"], inputs["b2b"]
    f1 = np.ascontiguousarray(np.asarray(feature1, dtype=np.float32))
    f2 = np.ascontiguousarray(np.asarray(feature2, dtype=np.float32))
    r_self = int(np.asarray(inputs["positive_range_self"]))
    r_tgt = int(np.asarray(inputs["positive_range_tgt"]))
    share_tgt = r_tgt == r_self

    nc = _get_nc(share_tgt)

    x2t_full = np.ascontiguousarray(f2.reshape(N, DIN2).T.astype(bf16))  # (192, 8192)
    wpk = _pack_consts(
        W1a, W2a, W1b, W2b, b1a, b2a, b1b, b2b,
        _cnt_inv(r_self), None if share_tgt else _cnt_inv(r_tgt),
    )
    common = {
        "wpk": wpk,
        "bms": _band_mask(r_self).astype(bf16),
    }
    if not share_tgt:
        common["bmt"] = _band_mask(r_tgt).astype(bf16)

    in_maps = []
    for c in range(NCORES):
        x1t = np.ascontiguousarray(
            f1[c * NB : (c + 1) * NB].reshape(NLOC, DIN1).T.astype(bf16)
        )  # (256, 1024)
        # rotate feature2^T columns so this core's rows come first
        x2t = np.ascontiguousarray(
            np.concatenate(
                [x2t_full[:, c * NLOC :], x2t_full[:, : c * NLOC]], axis=1
            )
        )
        in_maps.append({**common, "x1t": x1t, "x2t": x2t})

    res = run_bass_kernel_spmd(
        nc,
        in_maps,
        core_ids=list(range(NCORES)),
        trace=trace,
        **(trace_kwargs or {}),
    )

    pos = np.empty(N, dtype=np.float64)
    se = np.empty(N, dtype=np.float64)
    for c in range(NCORES):
        # column t holds local rows [t*128, (t+1)*128) in partitions
        p = res.results[c]["pos_out"]  # (128, NT)
        s = res.results[c]["se_out"]
        pos[c * NLOC : (c + 1) * NLOC] = p.T.reshape(NLOC)
        se[c * NLOC : (c + 1) * NLOC] = s.T.reshape(NLOC)

    neg = np.log(se) - np.log(float(N))
    loss = np.mean(-pos + neg)
    return np.array(loss, dtype=np.float32), res
